# revision 14
# baseline (speedup 1.0000x reference)
"""BitLinear forward on 8 Trainium2 NeuronCores.

out = (x_q @ w_q) * (beta * gamma)
  a      = mean(weight);  w_q = sign(weight - a)
  gamma  = max|x| per row; x_q = clip(x/(gamma+eps), -(1-eps), 1-eps)
  beta   = max|weight|

Sharding: data-parallel over rows of x (N=32768 -> 4096 rows/core),
weight (1024x1024) replicated; per-core scalar stats are computed
redundantly so no collectives are needed.

Kernel math notes:
  - Since QB == 1, (x_q @ w_q)*beta*gamma equals (x @ w_q)*beta *
    gamma/(gamma+eps) up to the +-(1-eps) clip; the clip and eps terms
    are < 1e-5 relative, far below the 16-bit rounding of the matmul.
    So the kernel computes (x @ w_q) scaled by beta; gamma is never
    materialized.
  - The sign is computed as (w >= a) - 0.5 = +-0.5 on the DVE (one
    two-op tensor_scalar); the missing factor 2 rides the final scale.
  - The output is stored UNSCALED in bf16 (bf16 is scale-invariant);
    beta ships out as a tiny [1,2] tensor and the host folds 2*beta
    into the bf16 -> f32 upcast it already performs.  This keeps the
    beta reduction entirely off the device critical path.
  - Features 0..FP8C*128 run as fp8-e4m3 DoubleRow matmuls (2 virtual
    k-rows per cycle, ~2x PE rate); the rest stay bf16.  Measured
    end-to-end scale-rel err 1.25e-2 vs the 2e-2 gate (bf16-only is
    3.3e-3; full fp8 would be 2.5e-2 and fails).

Schedule (per core; times approximate, from perfetto traces):
  - The weight mean gates the signs and therefore every matmul, so the
    4MiB weight load owns the HBM bandwidth: it arrives as 4 x 1MiB
    DMAs, two per HWDGE ring (a ring retires chunk DMAs no faster than
    ~2.2us each regardless of size, so 1MiB is the efficiency knee).
    x chunk 0 (the 2 ramp tiles) rides SWDGE concurrently; all other x
    chunks are data-gated behind the mean via token writes into their
    own DMA target slices (engine program order alone gets reordered
    by the Tile scheduler).
  - Per-chunk partial sums ride the group arrivals, split DVE
    (tensor_reduce, 1.2us) / ACT (accum_out copy, 1.4us).  The
    abs-maxes for beta run on DVE after the signs, token-gated so the
    scheduler cannot hoist them into the critical chain.
  - Signs are 16 half-chunk tensor_scalar ops on DVE (~0.47us each),
    h-major so the fp8 pair (which the DoubleRow matmul consumes
    first) lands first; the first TWO tiles' matmuls interleave with
    sign production (2 matmuls per arriving half = exactly the DVE
    production rate).
  - 48 warm-up matmuls run under the weight DMA so the HAM clock gate
    is at 8/8 when the real stream starts.
  - Steady state: 32 tiles x (2 DoubleRow + 12 bf16) matmuls, N=512,
    ~216ns each; ACT evacuates h0 and DVE h1 of each PSUM tile in
    parallel; half stores alternate the two HWDGE rings.
"""

import sys

import numpy as np

if "/opt/trn_rl_repo" not in sys.path:
    sys.path.insert(0, "/opt/trn_rl_repo")

N_CORES = 8
N_FEAT = 1024
N_OUT = 1024
P = 128
KC = N_FEAT // P  # 8 contraction chunks of 128
FP8C = 4  # leading chunks that run as fp8 DoubleRow (must be even)
KCB = KC - FP8C  # bf16 chunks
N_WARM = 40  # warm-up matmuls issued under the weight DMA
RAMP = 2  # tiles interleaved during sign production

_NC_CACHE = {}
_PATCHED = False


def _split_multi_waits(nc, max_waits=1):
    """The walrus build in this image rejects instructions carrying more
    than one sync-wait ("Too many sync wait commands").  Tile's semaphore
    assignment attaches one wait per producer proc, so hoist surplus waits
    onto NOP carrier instructions inserted immediately before the waiting
    instruction on the same engine (waits execute before the instruction
    body, so this preserves semantics exactly)."""
    import bass_rust

    for fn in nc.m.functions:
        for blk in fn.blocks:
            insts = blk.instructions  # live list
            i = 0
            while i < len(insts):
                ins = insts[i]
                si = getattr(ins, "sync_info", None)
                if si is None:
                    i += 1
                    continue
                waits = list(si.on_wait)
                if len(waits) <= max_waits:
                    i += 1
                    continue
                keep = waits[:max_waits]
                surplus = waits[max_waits:]
                si.on_wait = keep
                carriers = []
                cur_list = nc.cur_bb.bb.instructions
                for j in range(0, len(surplus), max_waits):
                    nop = nc.engines[ins.engine].nop(nofuse=True)
                    nop.ins.sync_info = bass_rust.SyncInfo(
                        on_wait=surplus[j : j + max_waits], on_update=[]
                    )
                    popped = cur_list.pop()
                    assert popped is nop.ins
                    carriers.append(nop.ins)
                for k, c in enumerate(carriers):
                    insts.insert(i + k, c)
                i += len(carriers) + 1


def _patch_tile_drain():
    global _PATCHED
    if _PATCHED:
        return
    _PATCHED = True
    import concourse.tile as tile

    orig = tile.TileContext._drain_and_barrier

    def patched(self, tick_clock, wait_clock):
        orig(self, tick_clock, wait_clock)
        _split_multi_waits(self.nc)

    tile.TileContext._drain_and_barrier = patched


def _build_nc(rows_per_core: int):
    import concourse.bass as bass
    import concourse.mybir as mybir
    import concourse.tile as tile

    _patch_tile_drain()

    f32 = mybir.dt.float32
    f32r = mybir.dt.float32r
    bf16 = mybir.dt.bfloat16
    fp8 = mybir.dt.float8e4
    DR = mybir.MatmulPerfMode.DoubleRow
    R = rows_per_core
    assert R % P == 0
    T = R // P

    nc = bass.Bass("TRN2", target_bir_lowering=False, debug=False)
    # xt8[p, c*R + r] = x[r, c*128 + p]          for c in [0, FP8C)
    # xt [p, c*R + r] = x[r, (FP8C+c)*128 + p]   for c in [0, KCB)
    x8_h = nc.declare_dram_parameter("xt8", [P, FP8C * R], fp8, isOutput=False)
    xt_h = nc.declare_dram_parameter("xt", [P, KCB * R], bf16, isOutput=False)
    # float32r = same bits as f32; only the PE's read path truncates to
    # fp22.  Typing the weight f32r end-to-end satisfies the BIR
    # verifier for the fp32r sum-matmuls; DVE/ACT consumers see the
    # full 32-bit values.
    w_h = nc.declare_dram_parameter("weight", [N_FEAT, N_OUT], f32r, isOutput=False)
    o_h = nc.declare_dram_parameter("out", [R, N_OUT], bf16, isOutput=True)
    b_h = nc.declare_dram_parameter("bout", [1, 2], f32, isOutput=True)

    x8_ap = x8_h[:, :].rearrange("p (c r) -> p c r", c=FP8C)
    xt_ap = xt_h[:, :].rearrange("p (c r) -> p c r", c=KCB)
    o_ap = o_h[:, :]
    # weight[c*128 + p, n] -> [p, c, n]
    w_ap = w_h[:, :].rearrange("(c p) n -> p c n", p=P)

    # x chunk row boundaries: chunk 0 = the RAMP tiles (ungated), rest
    # 512-row chunks gated behind the mean
    xb = [0, RAMP * P]
    for step in (256, 512, 512, 768, 768, 1024, 1024, 1024):
        if xb[-1] >= R:
            break
        xb.append(min(xb[-1] + step, R))
    assert xb[-1] == R
    n_xch = len(xb) - 1

    with tile.TileContext(nc) as tc:
        with (
            tc.tile_pool(name="wpool", bufs=1) as wpool,
            tc.tile_pool(name="xtpool", bufs=1) as xtpool,
            tc.tile_pool(name="opool", bufs=10) as opool,
            tc.tile_pool(name="pspool", bufs=3, space="PSUM") as pspool,
            tc.tile_pool(name="ps1pool", bufs=2, space="PSUM") as ps1pool,
        ):
            # ---- SBUF-resident tensors ----
            w32 = wpool.tile([P, KC, N_OUT], f32r, tag="w32")
            wq8 = wpool.tile([P, FP8C, N_OUT], fp8, tag="wq8")
            wq = wpool.tile([P, KCB, N_OUT], bf16, tag="wq")
            wmax = wpool.tile([P, KC], f32, tag="wmax")
            ssum = wpool.tile([P, 1], f32, tag="ssum")
            bmax = wpool.tile([P, 1], f32, tag="bmax")
            pack2 = wpool.tile([1, 2], f32, tag="pack2")
            ones128 = wpool.tile([P, P], f32r, tag="ones128")
            onesf = wpool.tile([P, P], f32, tag="onesf")
            stats = wpool.tile([P, 2], f32, tag="stats")
            token = wpool.tile([1, 1], bf16, tag="token")
            tok8 = wpool.tile([1, 1], fp8, tag="tok8")
            onesb = wpool.tile([P, 512], bf16, tag="onesb")
            x8 = xtpool.tile([P, FP8C, R], fp8, tag="x8")
            xt = xtpool.tile([P, KCB, R], bf16, tag="xt")

            nc.vector.memset(pack2, 0.0)
            nc.vector.memset(onesf, 1.0)
            nc.vector.tensor_copy(out=ones128, in_=onesf)
            nc.vector.memset(onesb, 1.0)

            # ---- weight DMA: 4 x 1MiB (2 chunks each), two per HWDGE
            # ring; x chunk 0 rides SWDGE concurrently ----
            w_engines = [nc.sync, nc.scalar]
            for g in range(4):
                w_engines[g % 2].dma_start(
                    out=w32[:, 2 * g : 2 * g + 2, :],
                    in_=w_ap[:, 2 * g : 2 * g + 2, :],
                )
            nc.gpsimd.dma_start(
                out=x8[:, :, 0 : xb[1]], in_=x8_ap[:, :, 0 : xb[1]]
            )
            nc.gpsimd.dma_start(
                out=xt[:, :, 0 : xb[1]], in_=xt_ap[:, :, 0 : xb[1]]
            )

            # ---- PE warm-up under the weight DMA: keeps the HAM clock
            # gate from parking at 4/8 (1.2GHz) before the real stream
            warm_ps = ps1pool.tile([P, 512], f32, tag="scratch")
            for _ in range(N_WARM):
                nc.tensor.matmul(
                    warm_ps, onesb[:, 0:P], onesb, start=True, stop=True
                )

            # ---- weight sum on the (otherwise idle) PE: 16 fp32r
            # ones-matmuls accumulate per-column sums of every 512-col
            # half into one PSUM bank (fp32r streams 1 col/cycle at
            # N=512; fp22 read truncation is symmetric over the +-
            # uniform weights, so the mean error is ~1e-9 relative --
            # far below the nearest-weight gap).  One DVE reduce then
            # yields the total, already replicated across partitions. ----
            sum_ps = ps1pool.tile([P, 512], f32, tag="scratch")
            i = 0
            for g in range(KC // 2):
                if g:
                    # fillers: keep the PE busy (HAM warm) while the next
                    # 1MiB weight group is still in flight
                    for _ in range(5):
                        nc.tensor.matmul(
                            warm_ps, onesb[:, 0:P], onesb,
                            start=True, stop=True,
                        )
                for c in (2 * g, 2 * g + 1):
                    for h in range(2):
                        nc.tensor.matmul(
                            sum_ps,
                            ones128,
                            w32[:, c, h * 512 : (h + 1) * 512],
                            start=(i == 0),
                            stop=(i == 2 * KC - 1),
                        )
                        i += 1
            nc.vector.tensor_reduce(
                ssum, sum_ps, axis=mybir.AxisListType.X, op=mybir.AluOpType.add
            )
            nc.vector.tensor_scalar_mul(
                stats[:, 0:1], ssum, 1.0 / float(N_FEAT * N_OUT)
            )
            mean_a = stats[:, 0:1]
            nc.vector.tensor_scalar_mul(
                stats[:, 1:2], ssum, -1.0 / float(N_FEAT * N_OUT)
            )
            neg_a = stats[:, 1:2]

            # gate the remaining x loads behind the full weight arrival:
            # tokens derived from ssum are written INTO each chunk's DMA
            # target slices, a WAW dependency the scheduler must honor
            nc.vector.tensor_copy(out=token, in_=ssum[0:1, 0:1])
            nc.vector.tensor_copy(out=tok8, in_=ssum[0:1, 0:1])
            for q in range(1, n_xch):
                nc.vector.tensor_copy(
                    out=x8[0:1, 0:1, xb[q] : xb[q] + 1], in_=tok8
                )
                nc.vector.tensor_copy(
                    out=xt[0:1, 0:1, xb[q] : xb[q] + 1], in_=token
                )

            # signs in 512-col halves, h-major, produced by BOTH engines
            # in parallel: DVE computes (w >= a) - 0.5 = +-0.5 for the
            # fp8 chunks (consumed first by the DoubleRow matmuls), ACT
            # computes sign(w - a) = +-1 for the bf16 chunks, whose x
            # features arrive pre-halved from the host
            for h in range(2):
                hs = slice(h * 512, (h + 1) * 512)
                for c in range(FP8C):
                    nc.vector.tensor_scalar(
                        out=wq8[:, c, hs],
                        in0=w32[:, c, hs],
                        scalar1=mean_a,
                        scalar2=0.5,
                        op0=mybir.AluOpType.is_ge,
                        op1=mybir.AluOpType.subtract,
                    )
            for h in range(2):
                hs = slice(h * 512, (h + 1) * 512)
                for c in range(FP8C, KC):
                    nc.scalar.activation(
                        out=wq[:, c - FP8C, hs],
                        in_=w32[:, c, hs],
                        func=mybir.ActivationFunctionType.Sign,
                        bias=neg_a, scale=1.0,
                    )

            # the gated x loads (both dtypes per row chunk), spread
            # round-robin over all three DMA queues
            x_queues = [nc.sync, nc.sync, nc.scalar, nc.gpsimd,
                        nc.sync, nc.scalar, nc.gpsimd, nc.scalar]
            for q in range(1, n_xch):
                eng = x_queues[(q - 1) % len(x_queues)]
                eng.dma_start(
                    out=x8[:, :, xb[q] : xb[q + 1]],
                    in_=x8_ap[:, :, xb[q] : xb[q + 1]],
                )
                eng.dma_start(
                    out=xt[:, :, xb[q] : xb[q + 1]],
                    in_=xt_ap[:, :, xb[q] : xb[q + 1]],
                )

            # ---- beta: entirely OFF the device critical path.  The
            # output is stored unscaled; beta ships out as a tiny tensor
            # and the HOST folds 2*beta into its f32 upcast.  Token
            # writes stop the scheduler hoisting these DVE reduces into
            # the critical chain above. ----
            nc.vector.tensor_copy(
                out=wmax[0:1, KC - 1 : KC],
                in_=wq8[0:1, FP8C - 1, N_OUT - 1 : N_OUT],
            )
            for c in range(KC):
                nc.vector.tensor_copy(
                    out=wmax[0:1, c : c + 1], in_=wmax[0:1, KC - 1 : KC]
                )
                nc.vector.tensor_reduce(
                    wmax[:, c : c + 1], w32[:, c, :],
                    axis=mybir.AxisListType.X, op=mybir.AluOpType.max,
                    apply_absolute_value=True,
                )
            nc.vector.tensor_reduce(
                bmax, wmax, axis=mybir.AxisListType.X, op=mybir.AluOpType.max
            )
            nc.gpsimd.tensor_reduce(
                pack2[:, 1:2], bmax, axis=mybir.AxisListType.C,
                op=mybir.AluOpType.max,
            )
            nc.gpsimd.dma_start(out=b_h[:, :], in_=pack2)

            def emit_tile_mms(groups):
                """groups: list of (psum_tile, t) pairs emitted
                interleaved per (h, k-group) so sign production feeds
                len(groups) matmuls per arriving half."""
                for h in range(2):
                    hs = slice(h * 512, (h + 1) * 512)
                    for gi in range(FP8C // 2 + KCB):
                        for ps, t in groups:
                            if gi < FP8C // 2:
                                nc.tensor.matmul(
                                    ps[:, hs],
                                    x8[
                                        :,
                                        2 * gi : 2 * gi + 2,
                                        t * P : (t + 1) * P,
                                    ],
                                    wq8[:, 2 * gi : 2 * gi + 2, hs],
                                    start=(gi == 0),
                                    stop=False,
                                    perf_mode=DR,
                                )
                            else:
                                cc = gi - FP8C // 2
                                nc.tensor.matmul(
                                    ps[:, hs],
                                    xt[:, cc, t * P : (t + 1) * P],
                                    wq[:, cc, hs],
                                    start=False,
                                    stop=(cc == KCB - 1),
                                )

            def emit_evac(t, ps):
                # ACT evacuates h0, DVE h1 in parallel (different PSUM
                # banks); half stores alternate the two HWDGE rings
                o = opool.tile([P, N_OUT], bf16, tag="o", name=f"o_{t}")
                nc.scalar.activation(
                    out=o[:, 0:512], in_=ps[:, 0:512],
                    func=mybir.ActivationFunctionType.Copy,
                    bias=0.0, scale=1.0,
                )
                nc.sync.dma_start(
                    out=o_ap[t * P : (t + 1) * P, 0:512], in_=o[:, 0:512]
                )
                nc.vector.tensor_copy(out=o[:, 512:1024], in_=ps[:, 512:1024])
                nc.scalar.dma_start(
                    out=o_ap[t * P : (t + 1) * P, 512:1024], in_=o[:, 512:1024]
                )

            # ---- ramp: first RAMP tiles interleaved with sign
            # production, then the steady stream ----
            assert T >= RAMP
            ramp_ps = [
                pspool.tile([P, N_OUT], f32, tag="ps", name=f"ps_i{t}")
                for t in range(RAMP)
            ]
            emit_tile_mms([(ramp_ps[t], t) for t in range(RAMP)])
            for t in range(RAMP):
                emit_evac(t, ramp_ps[t])

            for t in range(RAMP, T):
                ps = pspool.tile([P, N_OUT], f32, tag="ps")
                emit_tile_mms([(ps, t)])
                emit_evac(t, ps)

    return nc


def _get_nc(rows_per_core: int):
    if rows_per_core not in _NC_CACHE:
        _NC_CACHE[rows_per_core] = _build_nc(rows_per_core)
    return _NC_CACHE[rows_per_core]


def _prep_core_inputs(x, weight):
    """Host-side shard + layout: per-core feature-major xT, fp8 for the
    leading FP8C*128 features, bf16 for the rest."""
    import ml_dtypes

    n = x.shape[0]
    rpc = n // N_CORES
    kf = FP8C * P
    in_maps = []
    for i in range(N_CORES):
        xi = x[i * rpc : (i + 1) * rpc]
        x8 = xi[:, :kf].reshape(rpc, FP8C, P).transpose(2, 1, 0)
        x8 = np.ascontiguousarray(x8.astype(ml_dtypes.float8_e4m3fn))
        # the bf16 chunks' signs are +-1 on device, so their x is
        # pre-halved (exact exponent shift) to keep all contributions
        # at x * +-0.5
        xt = (xi[:, kf:] * np.float32(0.5)).reshape(rpc, KCB, P)
        xt = xt.transpose(2, 1, 0)
        xt = np.ascontiguousarray(xt.astype(ml_dtypes.bfloat16))
        in_maps.append(
            {
                "xt8": x8.reshape(P, FP8C * rpc),
                "xt": xt.reshape(P, KCB * rpc),
                "weight": weight,
            }
        )
    return in_maps, rpc


def run(x, weight, trace=False, trace_cores=None):
    """Run on 8 cores; returns (out, BassKernelResults)."""
    from concourse.bass_utils import run_bass_kernel_spmd

    x = np.ascontiguousarray(np.asarray(x, dtype=np.float32))
    weight = np.ascontiguousarray(np.asarray(weight, dtype=np.float32))
    n = x.shape[0]
    assert n % N_CORES == 0
    in_maps, rpc = _prep_core_inputs(x, weight)
    nc = _get_nc(rpc)
    kwargs = {}
    if trace:
        kwargs["trace"] = True
        if trace_cores is not None:
            kwargs["trace_cores"] = trace_cores
    res = run_bass_kernel_spmd(nc, in_maps, core_ids=list(range(N_CORES)), **kwargs)
    # signs on device are +-0.5 and the output is stored unscaled, so
    # the final scale is 2*beta, folded into the bf16 -> f32 upcast
    beta = float(np.asarray(res.results[0]["bout"], dtype=np.float32)[0, 1])
    out = np.concatenate([r["out"] for r in res.results], axis=0)
    out = np.asarray(out, dtype=np.float32) * np.float32(2.0 * beta)
    return out, res


def kernel(x, weight):
    out, _ = run(x, weight)
    return out


# revision 15
# speedup vs baseline: 1.1202x; 1.1202x over previous
"""BitLinear forward on 8 Trainium2 NeuronCores.

out = (x_q @ w_q) * (beta * gamma)
  a      = mean(weight);  w_q = sign(weight - a)
  gamma  = max|x| per row; x_q = clip(x/(gamma+eps), -(1-eps), 1-eps)
  beta   = max|weight|

Sharding: data-parallel over rows of x (N=32768 -> 4096 rows/core),
weight (1024x1024) replicated; per-core scalar stats are computed
redundantly so no collectives are needed.

Kernel math notes:
  - Since QB == 1, (x_q @ w_q)*beta*gamma equals (x @ w_q)*beta *
    gamma/(gamma+eps) up to the +-(1-eps) clip; the clip and eps terms
    are < 1e-5 relative, far below the 16-bit rounding of the matmul.
    So the kernel computes (x @ w_q) scaled by beta; gamma is never
    materialized.
  - The sign is computed as (w >= a) - 0.5 = +-0.5 on the DVE (one
    two-op tensor_scalar); the missing factor 2 rides the final scale.
  - The output is stored UNSCALED in bf16 (bf16 is scale-invariant);
    beta ships out as a tiny [1,2] tensor and the host folds 2*beta
    into the bf16 -> f32 upcast it already performs.  This keeps the
    beta reduction entirely off the device critical path.
  - Features 0..FP8C*128 run as fp8-e4m3 DoubleRow matmuls (2 virtual
    k-rows per cycle, ~2x PE rate); the rest stay bf16.  Measured
    end-to-end scale-rel err 1.25e-2 vs the 2e-2 gate (bf16-only is
    3.3e-3; full fp8 would be 2.5e-2 and fails).

Schedule (per core; times approximate, from perfetto traces):
  - The weight mean gates the signs and therefore every matmul, so the
    4MiB weight load owns the HBM bandwidth: it arrives as 4 x 1MiB
    DMAs, two per HWDGE ring (a ring retires chunk DMAs no faster than
    ~2.2us each regardless of size, so 1MiB is the efficiency knee).
    x chunk 0 (the 2 ramp tiles) rides SWDGE concurrently; all other x
    chunks are data-gated behind the mean via token writes into their
    own DMA target slices (engine program order alone gets reordered
    by the Tile scheduler).
  - Per-chunk partial sums ride the group arrivals, split DVE
    (tensor_reduce, 1.2us) / ACT (accum_out copy, 1.4us).  The
    abs-maxes for beta run on DVE after the signs, token-gated so the
    scheduler cannot hoist them into the critical chain.
  - Signs are 16 half-chunk tensor_scalar ops on DVE (~0.47us each),
    h-major so the fp8 pair (which the DoubleRow matmul consumes
    first) lands first; the first TWO tiles' matmuls interleave with
    sign production (2 matmuls per arriving half = exactly the DVE
    production rate).
  - 48 warm-up matmuls run under the weight DMA so the HAM clock gate
    is at 8/8 when the real stream starts.
  - Steady state: 32 tiles x (2 DoubleRow + 12 bf16) matmuls, N=512,
    ~216ns each; ACT evacuates h0 and DVE h1 of each PSUM tile in
    parallel; half stores alternate the two HWDGE rings.
"""

import sys

import numpy as np

if "/opt/trn_rl_repo" not in sys.path:
    sys.path.insert(0, "/opt/trn_rl_repo")

N_CORES = 8
N_FEAT = 1024
N_OUT = 1024
P = 128
KC = N_FEAT // P  # 8 contraction chunks of 128
FP8C = 4  # leading chunks that run as fp8 DoubleRow (must be even)
KCB = KC - FP8C  # bf16 chunks
N_WARM = 40  # warm-up matmuls issued under the weight DMA
RAMP = 2  # tiles interleaved during sign production

_NC_CACHE = {}
_PATCHED = False


def _split_multi_waits(nc, max_waits=1):
    """The walrus build in this image rejects instructions carrying more
    than one sync-wait ("Too many sync wait commands").  Tile's semaphore
    assignment attaches one wait per producer proc, so hoist surplus waits
    onto NOP carrier instructions inserted immediately before the waiting
    instruction on the same engine (waits execute before the instruction
    body, so this preserves semantics exactly)."""
    import bass_rust

    for fn in nc.m.functions:
        for blk in fn.blocks:
            insts = blk.instructions  # live list
            i = 0
            while i < len(insts):
                ins = insts[i]
                si = getattr(ins, "sync_info", None)
                if si is None:
                    i += 1
                    continue
                waits = list(si.on_wait)
                if len(waits) <= max_waits:
                    i += 1
                    continue
                keep = waits[:max_waits]
                surplus = waits[max_waits:]
                si.on_wait = keep
                carriers = []
                cur_list = nc.cur_bb.bb.instructions
                for j in range(0, len(surplus), max_waits):
                    nop = nc.engines[ins.engine].nop(nofuse=True)
                    nop.ins.sync_info = bass_rust.SyncInfo(
                        on_wait=surplus[j : j + max_waits], on_update=[]
                    )
                    popped = cur_list.pop()
                    assert popped is nop.ins
                    carriers.append(nop.ins)
                for k, c in enumerate(carriers):
                    insts.insert(i + k, c)
                i += len(carriers) + 1


def _patch_tile_drain():
    global _PATCHED
    if _PATCHED:
        return
    _PATCHED = True
    import concourse.tile as tile

    orig = tile.TileContext._drain_and_barrier

    def patched(self, tick_clock, wait_clock):
        orig(self, tick_clock, wait_clock)
        _split_multi_waits(self.nc)

    tile.TileContext._drain_and_barrier = patched


def _build_nc(rows_per_core: int):
    import concourse.bass as bass
    import concourse.mybir as mybir
    import concourse.tile as tile

    _patch_tile_drain()

    f32 = mybir.dt.float32
    f32r = mybir.dt.float32r
    bf16 = mybir.dt.bfloat16
    fp8 = mybir.dt.float8e4
    DR = mybir.MatmulPerfMode.DoubleRow
    R = rows_per_core
    assert R % P == 0
    T = R // P

    nc = bass.Bass("TRN2", target_bir_lowering=False, debug=False)
    # xt8[p, c*R + r] = x[r, c*128 + p]          for c in [0, FP8C)
    # xt [p, c*R + r] = x[r, (FP8C+c)*128 + p]   for c in [0, KCB)
    x8_h = nc.declare_dram_parameter("xt8", [P, FP8C * R], fp8, isOutput=False)
    xt_h = nc.declare_dram_parameter("xt", [P, KCB * R], bf16, isOutput=False)
    # float32r = same bits as f32; only the PE's read path truncates to
    # fp22.  Typing the weight f32r end-to-end satisfies the BIR
    # verifier for the fp32r sum-matmuls; DVE/ACT consumers see the
    # full 32-bit values.
    w_h = nc.declare_dram_parameter("weight", [N_FEAT, N_OUT], f32r, isOutput=False)
    o_h = nc.declare_dram_parameter("out", [R, N_OUT], bf16, isOutput=True)
    b_h = nc.declare_dram_parameter("bout", [1, 2], f32, isOutput=True)

    x8_ap = x8_h[:, :].rearrange("p (c r) -> p c r", c=FP8C)
    xt_ap = xt_h[:, :].rearrange("p (c r) -> p c r", c=KCB)
    o_ap = o_h[:, :]
    # weight[c*128 + p, n] -> [p, c, n]
    w_ap = w_h[:, :].rearrange("(c p) n -> p c n", p=P)

    # x chunk row boundaries: chunk 0 = the RAMP tiles (ungated), rest
    # 512-row chunks gated behind the mean
    xb = [0, RAMP * P]
    for step in (256, 512, 512, 768, 768, 1024, 1024, 1024):
        if xb[-1] >= R:
            break
        xb.append(min(xb[-1] + step, R))
    assert xb[-1] == R
    n_xch = len(xb) - 1

    with tile.TileContext(nc) as tc:
        with (
            tc.tile_pool(name="wpool", bufs=1) as wpool,
            tc.tile_pool(name="xtpool", bufs=1) as xtpool,
            tc.tile_pool(name="opool", bufs=10) as opool,
            tc.tile_pool(name="pspool", bufs=3, space="PSUM") as pspool,
            tc.tile_pool(name="ps1pool", bufs=2, space="PSUM") as ps1pool,
        ):
            # ---- SBUF-resident tensors ----
            w32 = wpool.tile([P, KC, N_OUT], f32r, tag="w32")
            wq8 = wpool.tile([P, FP8C, N_OUT], fp8, tag="wq8")
            wq = wpool.tile([P, KCB, N_OUT], bf16, tag="wq")
            wmax = wpool.tile([P, KC], f32, tag="wmax")
            ssum = wpool.tile([P, 1], f32, tag="ssum")
            bmax = wpool.tile([P, 1], f32, tag="bmax")
            pack2 = wpool.tile([1, 2], f32, tag="pack2")
            ones128 = wpool.tile([P, P], f32r, tag="ones128")
            onesf = wpool.tile([P, P], f32, tag="onesf")
            stats = wpool.tile([P, 2], f32, tag="stats")
            token = wpool.tile([1, 1], bf16, tag="token")
            tok8 = wpool.tile([1, 1], fp8, tag="tok8")
            onesb = wpool.tile([P, 512], bf16, tag="onesb")
            x8 = xtpool.tile([P, FP8C, R], fp8, tag="x8")
            xt = xtpool.tile([P, KCB, R], bf16, tag="xt")

            nc.vector.memset(pack2, 0.0)
            nc.vector.memset(onesf, 1.0)
            nc.vector.tensor_copy(out=ones128, in_=onesf)
            nc.vector.memset(onesb, 1.0)

            # ---- weight DMA: 4 x 1MiB (2 chunks each), two per HWDGE
            # ring; x chunk 0 rides SWDGE concurrently ----
            w_engines = [nc.sync, nc.scalar]
            for g in range(4):
                w_engines[g % 2].dma_start(
                    out=w32[:, 2 * g : 2 * g + 2, :],
                    in_=w_ap[:, 2 * g : 2 * g + 2, :],
                )
            nc.gpsimd.dma_start(
                out=x8[:, :, 0 : xb[1]], in_=x8_ap[:, :, 0 : xb[1]]
            )
            nc.gpsimd.dma_start(
                out=xt[:, :, 0 : xb[1]], in_=xt_ap[:, :, 0 : xb[1]]
            )

            # ---- PE warm-up under the weight DMA: keeps the HAM clock
            # gate from parking at 4/8 (1.2GHz) before the real stream
            warm_ps = ps1pool.tile([P, 512], f32, tag="scratch")
            for _ in range(N_WARM):
                nc.tensor.matmul(
                    warm_ps, onesb[:, 0:P], onesb, start=True, stop=True
                )

            # ---- weight sum on the (otherwise idle) PE: 16 fp32r
            # ones-matmuls accumulate per-column sums of every 512-col
            # half into one PSUM bank (fp32r streams 1 col/cycle at
            # N=512; fp22 read truncation is symmetric over the +-
            # uniform weights, so the mean error is ~1e-9 relative --
            # far below the nearest-weight gap).  One DVE reduce then
            # yields the total, already replicated across partitions. ----
            sum_ps = ps1pool.tile([P, 512], f32, tag="scratch")
            i = 0
            for g in range(KC // 2):
                if g:
                    # fillers: keep the PE busy (HAM warm) while the next
                    # 1MiB weight group is still in flight
                    for _ in range(5):
                        nc.tensor.matmul(
                            warm_ps, onesb[:, 0:P], onesb,
                            start=True, stop=True,
                        )
                for c in (2 * g, 2 * g + 1):
                    for h in range(2):
                        nc.tensor.matmul(
                            sum_ps,
                            ones128,
                            w32[:, c, h * 512 : (h + 1) * 512],
                            start=(i == 0),
                            stop=(i == 2 * KC - 1),
                        )
                        i += 1
            nc.vector.tensor_reduce(
                ssum, sum_ps, axis=mybir.AxisListType.X, op=mybir.AluOpType.add
            )
            nc.vector.tensor_scalar_mul(
                stats[:, 0:1], ssum, 1.0 / float(N_FEAT * N_OUT)
            )
            mean_a = stats[:, 0:1]

            # gate the remaining x loads behind the full weight arrival:
            # tokens derived from ssum are written INTO each chunk's DMA
            # target slices, a WAW dependency the scheduler must honor
            nc.vector.tensor_copy(out=token, in_=ssum[0:1, 0:1])
            nc.vector.tensor_copy(out=tok8, in_=ssum[0:1, 0:1])
            for q in range(1, n_xch):
                nc.vector.tensor_copy(
                    out=x8[0:1, 0:1, xb[q] : xb[q] + 1], in_=tok8
                )
                nc.vector.tensor_copy(
                    out=xt[0:1, 0:1, xb[q] : xb[q] + 1], in_=token
                )

            # signs on DVE in 512-col halves: wq' = (w >= a) - 0.5 =
            # +-0.5 (exact in fp8/bf16); h-major so the fp8 pairs that
            # the DoubleRow matmuls consume first land first
            for h in range(2):
                hs = slice(h * 512, (h + 1) * 512)
                for c in range(KC):
                    dst = wq8[:, c, hs] if c < FP8C else wq[:, c - FP8C, hs]
                    nc.vector.tensor_scalar(
                        out=dst,
                        in0=w32[:, c, hs],
                        scalar1=mean_a,
                        scalar2=0.5,
                        op0=mybir.AluOpType.is_ge,
                        op1=mybir.AluOpType.subtract,
                    )

            # the gated x loads (both dtypes per row chunk), spread
            # round-robin over all three DMA queues
            x_queues = [nc.sync, nc.sync, nc.scalar, nc.gpsimd,
                        nc.sync, nc.scalar, nc.gpsimd, nc.scalar]
            for q in range(1, n_xch):
                eng = x_queues[(q - 1) % len(x_queues)]
                eng.dma_start(
                    out=x8[:, :, xb[q] : xb[q + 1]],
                    in_=x8_ap[:, :, xb[q] : xb[q + 1]],
                )
                eng.dma_start(
                    out=xt[:, :, xb[q] : xb[q + 1]],
                    in_=xt_ap[:, :, xb[q] : xb[q + 1]],
                )

            def emit_tile_mms(groups):
                """groups: list of (psum_tile, t) pairs emitted
                interleaved per (h, k-group) so sign production feeds
                len(groups) matmuls per arriving half."""
                for h in range(2):
                    hs = slice(h * 512, (h + 1) * 512)
                    for gi in range(FP8C // 2 + KCB):
                        for ps, t in groups:
                            if gi < FP8C // 2:
                                nc.tensor.matmul(
                                    ps[:, hs],
                                    x8[
                                        :,
                                        2 * gi : 2 * gi + 2,
                                        t * P : (t + 1) * P,
                                    ],
                                    wq8[:, 2 * gi : 2 * gi + 2, hs],
                                    start=(gi == 0),
                                    stop=False,
                                    perf_mode=DR,
                                )
                            else:
                                cc = gi - FP8C // 2
                                nc.tensor.matmul(
                                    ps[:, hs],
                                    xt[:, cc, t * P : (t + 1) * P],
                                    wq[:, cc, hs],
                                    start=False,
                                    stop=(cc == KCB - 1),
                                )

            def emit_evac(t, ps):
                # ACT evacuates h0, DVE h1 in parallel (different PSUM
                # banks); half stores alternate the two HWDGE rings
                o = opool.tile([P, N_OUT], bf16, tag="o", name=f"o_{t}")
                nc.scalar.activation(
                    out=o[:, 0:512], in_=ps[:, 0:512],
                    func=mybir.ActivationFunctionType.Copy,
                    bias=0.0, scale=1.0,
                )
                nc.sync.dma_start(
                    out=o_ap[t * P : (t + 1) * P, 0:512], in_=o[:, 0:512]
                )
                nc.vector.tensor_copy(out=o[:, 512:1024], in_=ps[:, 512:1024])
                nc.scalar.dma_start(
                    out=o_ap[t * P : (t + 1) * P, 512:1024], in_=o[:, 512:1024]
                )

            # ---- ramp: first RAMP tiles interleaved with sign
            # production, then the steady stream ----
            assert T >= RAMP
            ramp_ps = [
                pspool.tile([P, N_OUT], f32, tag="ps", name=f"ps_i{t}")
                for t in range(RAMP)
            ]
            emit_tile_mms([(ramp_ps[t], t) for t in range(RAMP)])
            for t in range(RAMP):
                emit_evac(t, ramp_ps[t])

            # beta: entirely OFF the device critical path -- one DVE
            # abs-max per few tiles rides the evac stream's idle time
            # (emitting them before the evacs would block PSUM recycling
            # behind 10us of reduces and stall the PE).  The output is
            # stored unscaled; beta ships out as a tiny tensor and the
            # HOST folds 2*beta into its f32 upcast.
            beta_work = [
                lambda c=c: nc.vector.tensor_reduce(
                    wmax[:, c : c + 1], w32[:, c, :],
                    axis=mybir.AxisListType.X, op=mybir.AluOpType.max,
                    apply_absolute_value=True,
                )
                for c in range(KC)
            ]
            beta_work.append(
                lambda: nc.vector.tensor_reduce(
                    bmax, wmax, axis=mybir.AxisListType.X,
                    op=mybir.AluOpType.max,
                )
            )
            beta_work.append(
                lambda: (
                    nc.gpsimd.tensor_reduce(
                        pack2[:, 1:2], bmax, axis=mybir.AxisListType.C,
                        op=mybir.AluOpType.max,
                    ),
                    nc.gpsimd.dma_start(out=b_h[:, :], in_=pack2),
                )
            )
            for t in range(RAMP, T):
                ps = pspool.tile([P, N_OUT], f32, tag="ps")
                emit_tile_mms([(ps, t)])
                emit_evac(t, ps)
                if t % 2 == 1 and beta_work:
                    beta_work.pop(0)()

    return nc


def _get_nc(rows_per_core: int):
    if rows_per_core not in _NC_CACHE:
        _NC_CACHE[rows_per_core] = _build_nc(rows_per_core)
    return _NC_CACHE[rows_per_core]


def _prep_core_inputs(x, weight):
    """Host-side shard + layout: per-core feature-major xT, fp8 for the
    leading FP8C*128 features, bf16 for the rest."""
    import ml_dtypes

    n = x.shape[0]
    rpc = n // N_CORES
    kf = FP8C * P
    in_maps = []
    for i in range(N_CORES):
        xi = x[i * rpc : (i + 1) * rpc]
        x8 = xi[:, :kf].reshape(rpc, FP8C, P).transpose(2, 1, 0)
        x8 = np.ascontiguousarray(x8.astype(ml_dtypes.float8_e4m3fn))
        xt = xi[:, kf:].reshape(rpc, KCB, P).transpose(2, 1, 0)
        xt = np.ascontiguousarray(xt.astype(ml_dtypes.bfloat16))
        in_maps.append(
            {
                "xt8": x8.reshape(P, FP8C * rpc),
                "xt": xt.reshape(P, KCB * rpc),
                "weight": weight,
            }
        )
    return in_maps, rpc


def run(x, weight, trace=False, trace_cores=None):
    """Run on 8 cores; returns (out, BassKernelResults)."""
    from concourse.bass_utils import run_bass_kernel_spmd

    x = np.ascontiguousarray(np.asarray(x, dtype=np.float32))
    weight = np.ascontiguousarray(np.asarray(weight, dtype=np.float32))
    n = x.shape[0]
    assert n % N_CORES == 0
    in_maps, rpc = _prep_core_inputs(x, weight)
    nc = _get_nc(rpc)
    kwargs = {}
    if trace:
        kwargs["trace"] = True
        if trace_cores is not None:
            kwargs["trace_cores"] = trace_cores
    res = run_bass_kernel_spmd(nc, in_maps, core_ids=list(range(N_CORES)), **kwargs)
    # signs on device are +-0.5 and the output is stored unscaled, so
    # the final scale is 2*beta, folded into the bf16 -> f32 upcast
    beta = float(np.asarray(res.results[0]["bout"], dtype=np.float32)[0, 1])
    out = np.concatenate([r["out"] for r in res.results], axis=0)
    out = np.asarray(out, dtype=np.float32) * np.float32(2.0 * beta)
    return out, res


def kernel(x, weight):
    out, _ = run(x, weight)
    return out


# revision 16
# speedup vs baseline: 1.1844x; 1.0573x over previous
"""BitLinear forward on 8 Trainium2 NeuronCores.

out = (x_q @ w_q) * (beta * gamma)
  a      = mean(weight);  w_q = sign(weight - a)
  gamma  = max|x| per row; x_q = clip(x/(gamma+eps), -(1-eps), 1-eps)
  beta   = max|weight|

Sharding: data-parallel over rows of x (N=32768 -> 4096 rows/core),
weight (1024x1024) replicated; per-core scalar stats are computed
redundantly so no collectives are needed.

Kernel math notes:
  - Since QB == 1, (x_q @ w_q)*beta*gamma equals (x @ w_q)*beta *
    gamma/(gamma+eps) up to the +-(1-eps) clip; the clip and eps terms
    are < 1e-5 relative, far below the 16-bit rounding of the matmul.
    So the kernel computes (x @ w_q) scaled by beta; gamma is never
    materialized.
  - The sign is computed as (w >= a) - 0.5 = +-0.5 on the DVE (one
    two-op tensor_scalar); the missing factor 2 rides the final scale.
  - The output is stored UNSCALED in bf16 (bf16 is scale-invariant);
    beta ships out as a tiny [1,2] tensor and the host folds 2*beta
    into the bf16 -> f32 upcast it already performs.  This keeps the
    beta reduction entirely off the device critical path.
  - Features 0..FP8C*128 run as fp8-e4m3 DoubleRow matmuls (2 virtual
    k-rows per cycle, ~2x PE rate); the rest stay bf16.  Measured
    end-to-end scale-rel err 1.25e-2 vs the 2e-2 gate (bf16-only is
    3.3e-3; full fp8 would be 2.5e-2 and fails).

Schedule (per core; times approximate, from perfetto traces):
  - The weight mean gates the signs and therefore every matmul, so the
    4MiB weight load owns the HBM bandwidth: it arrives as 4 x 1MiB
    DMAs, two per HWDGE ring (a ring retires chunk DMAs no faster than
    ~2.2us each regardless of size, so 1MiB is the efficiency knee).
    x chunk 0 (the 2 ramp tiles) rides SWDGE concurrently; all other x
    chunks are data-gated behind the mean via token writes into their
    own DMA target slices (engine program order alone gets reordered
    by the Tile scheduler).
  - Per-chunk partial sums ride the group arrivals, split DVE
    (tensor_reduce, 1.2us) / ACT (accum_out copy, 1.4us).  The
    abs-maxes for beta run on DVE after the signs, token-gated so the
    scheduler cannot hoist them into the critical chain.
  - Signs are 16 half-chunk tensor_scalar ops on DVE (~0.47us each),
    h-major so the fp8 pair (which the DoubleRow matmul consumes
    first) lands first; the first TWO tiles' matmuls interleave with
    sign production (2 matmuls per arriving half = exactly the DVE
    production rate).
  - 48 warm-up matmuls run under the weight DMA so the HAM clock gate
    is at 8/8 when the real stream starts.
  - Steady state: 32 tiles x (2 DoubleRow + 12 bf16) matmuls, N=512,
    ~216ns each; ACT evacuates h0 and DVE h1 of each PSUM tile in
    parallel; half stores alternate the two HWDGE rings.
"""

import sys

import numpy as np

if "/opt/trn_rl_repo" not in sys.path:
    sys.path.insert(0, "/opt/trn_rl_repo")

N_CORES = 8
N_FEAT = 1024
N_OUT = 1024
P = 128
KC = N_FEAT // P  # 8 contraction chunks of 128
FP8C = 4  # leading chunks that run as fp8 DoubleRow (must be even)
KCB = KC - FP8C  # bf16 chunks
N_WARM = 40  # warm-up matmuls issued under the weight DMA
RAMP = 2  # tiles interleaved during sign production

_NC_CACHE = {}
_PATCHED = False


def _split_multi_waits(nc, max_waits=1):
    """The walrus build in this image rejects instructions carrying more
    than one sync-wait ("Too many sync wait commands").  Tile's semaphore
    assignment attaches one wait per producer proc, so hoist surplus waits
    onto NOP carrier instructions inserted immediately before the waiting
    instruction on the same engine (waits execute before the instruction
    body, so this preserves semantics exactly)."""
    import bass_rust

    for fn in nc.m.functions:
        for blk in fn.blocks:
            insts = blk.instructions  # live list
            i = 0
            while i < len(insts):
                ins = insts[i]
                si = getattr(ins, "sync_info", None)
                if si is None:
                    i += 1
                    continue
                waits = list(si.on_wait)
                if len(waits) <= max_waits:
                    i += 1
                    continue
                keep = waits[:max_waits]
                surplus = waits[max_waits:]
                si.on_wait = keep
                carriers = []
                cur_list = nc.cur_bb.bb.instructions
                for j in range(0, len(surplus), max_waits):
                    nop = nc.engines[ins.engine].nop(nofuse=True)
                    nop.ins.sync_info = bass_rust.SyncInfo(
                        on_wait=surplus[j : j + max_waits], on_update=[]
                    )
                    popped = cur_list.pop()
                    assert popped is nop.ins
                    carriers.append(nop.ins)
                for k, c in enumerate(carriers):
                    insts.insert(i + k, c)
                i += len(carriers) + 1


def _patch_tile_drain():
    global _PATCHED
    if _PATCHED:
        return
    _PATCHED = True
    import concourse.tile as tile

    orig = tile.TileContext._drain_and_barrier

    def patched(self, tick_clock, wait_clock):
        orig(self, tick_clock, wait_clock)
        _split_multi_waits(self.nc)

    tile.TileContext._drain_and_barrier = patched


def _build_nc(rows_per_core: int):
    import concourse.bass as bass
    import concourse.mybir as mybir
    import concourse.tile as tile

    _patch_tile_drain()

    f32 = mybir.dt.float32
    f32r = mybir.dt.float32r
    bf16 = mybir.dt.bfloat16
    fp8 = mybir.dt.float8e4
    DR = mybir.MatmulPerfMode.DoubleRow
    R = rows_per_core
    assert R % P == 0
    T = R // P

    nc = bass.Bass("TRN2", target_bir_lowering=False, debug=False)
    # xt8[p, c*R + r] = x[r, c*128 + p]          for c in [0, FP8C)
    # xt [p, c*R + r] = x[r, (FP8C+c)*128 + p]   for c in [0, KCB)
    x8_h = nc.declare_dram_parameter("xt8", [P, FP8C * R], fp8, isOutput=False)
    xt_h = nc.declare_dram_parameter("xt", [P, KCB * R], bf16, isOutput=False)
    # float32r = same bits as f32; only the PE's read path truncates to
    # fp22.  Typing the weight f32r end-to-end satisfies the BIR
    # verifier for the fp32r sum-matmuls; DVE/ACT consumers see the
    # full 32-bit values.
    w_h = nc.declare_dram_parameter("weight", [N_FEAT, N_OUT], f32r, isOutput=False)
    o_h = nc.declare_dram_parameter("out", [R, N_OUT], bf16, isOutput=True)
    b_h = nc.declare_dram_parameter("bout", [1, 2], f32, isOutput=True)

    x8_ap = x8_h[:, :].rearrange("p (c r) -> p c r", c=FP8C)
    xt_ap = xt_h[:, :].rearrange("p (c r) -> p c r", c=KCB)
    o_ap = o_h[:, :]
    # weight[c*128 + p, n] -> [p, c, n]
    w_ap = w_h[:, :].rearrange("(c p) n -> p c n", p=P)

    # x chunk row boundaries: chunk 0 = the RAMP tiles (ungated), rest
    # 512-row chunks gated behind the mean
    xb = [0, RAMP * P]
    for step in (256, 256, 512, 512, 768, 768, 1024, 1024):
        if xb[-1] >= R:
            break
        xb.append(min(xb[-1] + step, R))
    assert xb[-1] == R
    n_xch = len(xb) - 1

    with tile.TileContext(nc) as tc:
        with (
            tc.tile_pool(name="wpool", bufs=1) as wpool,
            tc.tile_pool(name="xtpool", bufs=1) as xtpool,
            tc.tile_pool(name="opool", bufs=10) as opool,
            tc.tile_pool(name="pspool", bufs=3, space="PSUM") as pspool,
            tc.tile_pool(name="ps1pool", bufs=2, space="PSUM") as ps1pool,
        ):
            # ---- SBUF-resident tensors ----
            w32 = wpool.tile([P, KC, N_OUT], f32r, tag="w32")
            wq8 = wpool.tile([P, FP8C, N_OUT], fp8, tag="wq8")
            wq = wpool.tile([P, KCB, N_OUT], bf16, tag="wq")
            wmax = wpool.tile([P, KC], f32, tag="wmax")
            ssum = wpool.tile([P, 1], f32, tag="ssum")
            bmax = wpool.tile([P, 1], f32, tag="bmax")
            pack2 = wpool.tile([1, 2], f32, tag="pack2")
            ones128 = wpool.tile([P, P], f32r, tag="ones128")
            onesf = wpool.tile([P, P], f32, tag="onesf")
            stats = wpool.tile([P, 2], f32, tag="stats")
            token = wpool.tile([1, 1], bf16, tag="token")
            tok8 = wpool.tile([1, 1], fp8, tag="tok8")
            onesb = wpool.tile([P, 512], bf16, tag="onesb")
            x8 = xtpool.tile([P, FP8C, R], fp8, tag="x8")
            xt = xtpool.tile([P, KCB, R], bf16, tag="xt")

            nc.vector.memset(pack2, 0.0)
            nc.vector.memset(onesf, 1.0)
            nc.vector.tensor_copy(out=ones128, in_=onesf)
            nc.vector.memset(onesb, 1.0)

            # ---- weight DMA: 4 x 1MiB (2 chunks each), two per HWDGE
            # ring; x chunk 0 rides SWDGE concurrently ----
            w_engines = [nc.sync, nc.scalar]
            for g in range(4):
                w_engines[g % 2].dma_start(
                    out=w32[:, 2 * g : 2 * g + 2, :],
                    in_=w_ap[:, 2 * g : 2 * g + 2, :],
                )
            nc.gpsimd.dma_start(
                out=x8[:, :, 0 : xb[1]], in_=x8_ap[:, :, 0 : xb[1]]
            )
            nc.gpsimd.dma_start(
                out=xt[:, :, 0 : xb[1]], in_=xt_ap[:, :, 0 : xb[1]]
            )

            # ---- PE warm-up under the weight DMA: keeps the HAM clock
            # gate from parking at 4/8 (1.2GHz) before the real stream
            warm_ps = ps1pool.tile([P, 512], f32, tag="scratch")
            for _ in range(N_WARM):
                nc.tensor.matmul(
                    warm_ps, onesb[:, 0:P], onesb, start=True, stop=True
                )

            # ---- weight sum on the (otherwise idle) PE: 16 fp32r
            # ones-matmuls accumulate per-column sums of every 512-col
            # half into one PSUM bank (fp32r streams 1 col/cycle at
            # N=512; fp22 read truncation is symmetric over the +-
            # uniform weights, so the mean error is ~1e-9 relative --
            # far below the nearest-weight gap).  One DVE reduce then
            # yields the total, already replicated across partitions. ----
            sum_ps = ps1pool.tile([P, 512], f32, tag="scratch")
            i = 0
            for c in range(KC):
                for h in range(2):
                    nc.tensor.matmul(
                        sum_ps,
                        ones128,
                        w32[:, c, h * 512 : (h + 1) * 512],
                        start=(i == 0),
                        stop=(i == 2 * KC - 1),
                    )
                    i += 1
            nc.vector.tensor_reduce(
                ssum, sum_ps, axis=mybir.AxisListType.X, op=mybir.AluOpType.add
            )
            nc.vector.tensor_scalar_mul(
                stats[:, 0:1], ssum, 1.0 / float(N_FEAT * N_OUT)
            )
            mean_a = stats[:, 0:1]

            # gate the remaining x loads behind the full weight arrival:
            # tokens derived from ssum are written INTO each chunk's DMA
            # target slices, a WAW dependency the scheduler must honor
            nc.vector.tensor_copy(out=token, in_=ssum[0:1, 0:1])
            nc.vector.tensor_copy(out=tok8, in_=ssum[0:1, 0:1])
            for q in range(1, n_xch):
                nc.vector.tensor_copy(
                    out=x8[0:1, 0:1, xb[q] : xb[q] + 1], in_=tok8
                )
                nc.vector.tensor_copy(
                    out=xt[0:1, 0:1, xb[q] : xb[q] + 1], in_=token
                )

            # signs on DVE in 512-col halves: wq' = (w >= a) - 0.5 =
            # +-0.5 (exact in fp8/bf16); h-major so the fp8 pairs that
            # the DoubleRow matmuls consume first land first
            for h in range(2):
                hs = slice(h * 512, (h + 1) * 512)
                for c in range(KC):
                    dst = wq8[:, c, hs] if c < FP8C else wq[:, c - FP8C, hs]
                    nc.vector.tensor_scalar(
                        out=dst,
                        in0=w32[:, c, hs],
                        scalar1=mean_a,
                        scalar2=0.5,
                        op0=mybir.AluOpType.is_ge,
                        op1=mybir.AluOpType.subtract,
                    )

            # the gated x loads (both dtypes per row chunk), spread
            # round-robin over all three DMA queues
            x_queues = [nc.sync, nc.scalar, nc.sync, nc.scalar,
                        nc.gpsimd, nc.sync, nc.scalar, nc.gpsimd]
            for q in range(1, n_xch):
                eng = x_queues[(q - 1) % len(x_queues)]
                eng.dma_start(
                    out=x8[:, :, xb[q] : xb[q + 1]],
                    in_=x8_ap[:, :, xb[q] : xb[q + 1]],
                )
                eng.dma_start(
                    out=xt[:, :, xb[q] : xb[q + 1]],
                    in_=xt_ap[:, :, xb[q] : xb[q + 1]],
                )

            def emit_tile_mms(groups):
                """groups: list of (psum_tile, t) pairs emitted
                interleaved per (h, k-group) so sign production feeds
                len(groups) matmuls per arriving half."""
                for h in range(2):
                    hs = slice(h * 512, (h + 1) * 512)
                    for gi in range(FP8C // 2 + KCB):
                        for ps, t in groups:
                            if gi < FP8C // 2:
                                nc.tensor.matmul(
                                    ps[:, hs],
                                    x8[
                                        :,
                                        2 * gi : 2 * gi + 2,
                                        t * P : (t + 1) * P,
                                    ],
                                    wq8[:, 2 * gi : 2 * gi + 2, hs],
                                    start=(gi == 0),
                                    stop=False,
                                    perf_mode=DR,
                                )
                            else:
                                cc = gi - FP8C // 2
                                nc.tensor.matmul(
                                    ps[:, hs],
                                    xt[:, cc, t * P : (t + 1) * P],
                                    wq[:, cc, hs],
                                    start=False,
                                    stop=(cc == KCB - 1),
                                )

            def emit_evac(t, ps):
                # ACT evacuates h0, DVE h1 in parallel (different PSUM
                # banks); half stores alternate the two HWDGE rings
                o = opool.tile([P, N_OUT], bf16, tag="o", name=f"o_{t}")
                nc.scalar.activation(
                    out=o[:, 0:512], in_=ps[:, 0:512],
                    func=mybir.ActivationFunctionType.Copy,
                    bias=0.0, scale=1.0,
                )
                nc.sync.dma_start(
                    out=o_ap[t * P : (t + 1) * P, 0:512], in_=o[:, 0:512]
                )
                nc.vector.tensor_copy(out=o[:, 512:1024], in_=ps[:, 512:1024])
                nc.scalar.dma_start(
                    out=o_ap[t * P : (t + 1) * P, 512:1024], in_=o[:, 512:1024]
                )

            # ---- ramp: first RAMP tiles interleaved with sign
            # production, then the steady stream ----
            assert T >= RAMP
            ramp_ps = [
                pspool.tile([P, N_OUT], f32, tag="ps", name=f"ps_i{t}")
                for t in range(RAMP)
            ]
            emit_tile_mms([(ramp_ps[t], t) for t in range(RAMP)])
            for t in range(RAMP):
                emit_evac(t, ramp_ps[t])

            # beta: entirely OFF the device critical path -- one DVE
            # abs-max per few tiles rides the evac stream's idle time
            # (emitting them before the evacs would block PSUM recycling
            # behind 10us of reduces and stall the PE).  The output is
            # stored unscaled; beta ships out as a tiny tensor and the
            # HOST folds 2*beta into its f32 upcast.
            beta_work = [
                lambda c=c: nc.vector.tensor_reduce(
                    wmax[:, c : c + 1], w32[:, c, :],
                    axis=mybir.AxisListType.X, op=mybir.AluOpType.max,
                    apply_absolute_value=True,
                )
                for c in range(KC)
            ]
            beta_work.append(
                lambda: nc.vector.tensor_reduce(
                    bmax, wmax, axis=mybir.AxisListType.X,
                    op=mybir.AluOpType.max,
                )
            )
            beta_work.append(
                lambda: (
                    nc.gpsimd.tensor_reduce(
                        pack2[:, 1:2], bmax, axis=mybir.AxisListType.C,
                        op=mybir.AluOpType.max,
                    ),
                    nc.gpsimd.dma_start(out=b_h[:, :], in_=pack2),
                )
            )
            for t in range(RAMP, T):
                ps = pspool.tile([P, N_OUT], f32, tag="ps")
                emit_tile_mms([(ps, t)])
                emit_evac(t, ps)
                if t % 2 == 1 and beta_work:
                    beta_work.pop(0)()

    return nc


def _get_nc(rows_per_core: int):
    if rows_per_core not in _NC_CACHE:
        _NC_CACHE[rows_per_core] = _build_nc(rows_per_core)
    return _NC_CACHE[rows_per_core]


def _prep_core_inputs(x, weight):
    """Host-side shard + layout: per-core feature-major xT, fp8 for the
    leading FP8C*128 features, bf16 for the rest."""
    import ml_dtypes

    n = x.shape[0]
    rpc = n // N_CORES
    kf = FP8C * P
    in_maps = []
    for i in range(N_CORES):
        xi = x[i * rpc : (i + 1) * rpc]
        x8 = xi[:, :kf].reshape(rpc, FP8C, P).transpose(2, 1, 0)
        x8 = np.ascontiguousarray(x8.astype(ml_dtypes.float8_e4m3fn))
        xt = xi[:, kf:].reshape(rpc, KCB, P).transpose(2, 1, 0)
        xt = np.ascontiguousarray(xt.astype(ml_dtypes.bfloat16))
        in_maps.append(
            {
                "xt8": x8.reshape(P, FP8C * rpc),
                "xt": xt.reshape(P, KCB * rpc),
                "weight": weight,
            }
        )
    return in_maps, rpc


def run(x, weight, trace=False, trace_cores=None):
    """Run on 8 cores; returns (out, BassKernelResults)."""
    from concourse.bass_utils import run_bass_kernel_spmd

    x = np.ascontiguousarray(np.asarray(x, dtype=np.float32))
    weight = np.ascontiguousarray(np.asarray(weight, dtype=np.float32))
    n = x.shape[0]
    assert n % N_CORES == 0
    in_maps, rpc = _prep_core_inputs(x, weight)
    nc = _get_nc(rpc)
    kwargs = {}
    if trace:
        kwargs["trace"] = True
        if trace_cores is not None:
            kwargs["trace_cores"] = trace_cores
    res = run_bass_kernel_spmd(nc, in_maps, core_ids=list(range(N_CORES)), **kwargs)
    # signs on device are +-0.5 and the output is stored unscaled, so
    # the final scale is 2*beta, folded into the bf16 -> f32 upcast
    beta = float(np.asarray(res.results[0]["bout"], dtype=np.float32)[0, 1])
    out = np.concatenate([r["out"] for r in res.results], axis=0)
    out = np.asarray(out, dtype=np.float32) * np.float32(2.0 * beta)
    return out, res


def kernel(x, weight):
    out, _ = run(x, weight)
    return out
